# revision 1
# baseline (speedup 1.0000x reference)
"""Trainium2 Bass kernel for nn_BinaryPathEncoder.

Math: out[n] = prod_{k} W_{b_k(pos_n)}^T  (product over the binary digits of
pos_n below its leading 1; W_0/W_1 = expm(herm_b), pad -> identity).

Let G_b = W_b^T = expm(-herm_b), M(h) = G_{b_0(h)} @ G_{b_1(h)} @ ... .
Split pos = hi*256 + lo:
  hi >= 1:  out = A8(lo) @ B(hi)   where A8(m) = 8-bit all-valid product,
                                          B(h) = M(h)  (h < 256)
  hi == 0:  out = M(pos) = B[pos]  = I @ B[pos]

Device (SPMD, identical program on 8 cores; per-core data differs):
  - expm via scaling-squaring Taylor (matmul-only, no solves)
  - builds A2/A4 doubling tables, the 256-entry B table (SBUF), and the
    257-entry stationary table STAT = [A8^T entries; identity] (DRAM)
  - position loop: per block of 4 positions sharing one stationary entry:
    stationary staged by dynamic-offset DMA from STAT, 4 moving operands
    staged from the SBUF B table, 4 matmuls into one PSUM bank, DVE evac,
    batched output DMA.
Host: computes lo/hi, packs positions into blocks (padded), distributes
blocks over cores, scatters results back to original order.
"""

import contextlib
import os

import numpy as np

import concourse.bass as bass
import concourse.bacc as bacc
import concourse.mybir as mybir
import concourse.tile as tile
import concourse.tile_utils as tile_utils
tile_utils.max_sbuf_usage = 206 * 1024
from concourse.bass_utils import run_bass_kernel_spmd
from concourse.masks import make_identity

FP = mybir.dt.float32
I32 = mybir.dt.int32
P = 128
NCORES = 8
S_EXP = 5          # scaling-squaring: X = -H / 2^S_EXP
ORDER = 18         # Taylor order
NB = 256           # B-table entries (hi < 256)
IDENT_ENTRY = 256  # stationary-table entry holding the identity

# mover engine per position-in-block: how the 4 moving operands get staged
#   "sync"/"gpsimd": dyn-offset DMA from the DRAM B copy
#   "vector"/"scalar": dyn-offset compute-engine copy from the SBUF B table
#   "gpsimd_sb": gpsimd SBUF->SBUF dma from the SBUF B table
MOVERS = tuple(os.environ.get("MOVERS", "sync,gpsimd,sync,gpsimd").split(","))
NEED_BDRAM = any(m in ("sync", "gpsimd") for m in MOVERS)

_prog_cache = {}
_last_ctx = None


def _mm(nc, out, lhsT, rhs):
    nc.tensor.matmul(out, lhsT=lhsT, rhs=rhs, start=True, stop=True)


def _build_expm(nc, consts, psump, scratch, praw, ident):
    """Return (G, GT) tile pairs: G_b = expm(-H_b), GT_b = G_b^T."""
    Gs, GTs = [], []
    for b in range(2):
        pb = praw[:, b, :]
        ps_t = psump.tile([P, 512], FP, tag="pos")
        nc.tensor.transpose(out=ps_t[:, :P], in_=pb, identity=ident[:])
        xt = consts.tile([P, P], FP, tag=f"xt{b}")
        # XT = (P - P^T)/2^s ;  lhsT=XT gives out = (-H/2^s) @ rhs since H^T=-H
        nc.vector.tensor_tensor(
            out=xt[:], in0=pb, in1=ps_t[:, :P], op=mybir.AluOpType.subtract
        )
        nc.vector.tensor_scalar_mul(xt[:], xt[:], 1.0 / (1 << S_EXP))

        t_cur = scratch.tile([P, P], FP, tag="tay")
        nc.vector.tensor_copy(t_cur[:], ident[:])
        for k in range(ORDER, 0, -1):
            ps = psump.tile([P, 512], FP, tag="pos")
            _mm(nc, ps[:, :P], xt[:], t_cur[:])
            t_nxt = scratch.tile([P, P], FP, tag="tay")
            nc.vector.tensor_scalar_mul(t_nxt[:], ps[:, :P], 1.0 / k)
            nc.vector.tensor_add(t_nxt[:], t_nxt[:], ident[:])
            t_cur = t_nxt
        # U = T^T
        ps_u = psump.tile([P, 512], FP, tag="pos")
        nc.tensor.transpose(out=ps_u[:, :P], in_=t_cur[:], identity=ident[:])
        u_cur = scratch.tile([P, P], FP, tag="tayu")
        nc.vector.tensor_copy(u_cur[:], ps_u[:, :P])
        for _ in range(S_EXP):
            ps1 = psump.tile([P, 512], FP, tag="pos")
            ps2 = psump.tile([P, 512], FP, tag="pos")
            _mm(nc, ps1[:, :P], u_cur[:], t_cur[:])   # T' = T @ T
            _mm(nc, ps2[:, :P], t_cur[:], u_cur[:])   # U' = (T@T)^T
            t_cur = scratch.tile([P, P], FP, tag="tay")
            u_cur = scratch.tile([P, P], FP, tag="tayu")
            nc.vector.tensor_copy(t_cur[:], ps1[:, :P])
            nc.vector.tensor_copy(u_cur[:], ps2[:, :P])
        g = consts.tile([P, P], FP, tag=f"g{b}")
        gt = consts.tile([P, P], FP, tag=f"gt{b}")
        nc.vector.tensor_copy(g[:], t_cur[:])
        nc.vector.tensor_copy(gt[:], u_cur[:])
        Gs.append(g)
        GTs.append(gt)
    return Gs, GTs


def build_program(n16, n4):
    nblk = n16 + n4
    nslots = n16 * 16 + n4 * 4
    nc = bacc.Bacc("TRN2", target_bir_lowering=False, debug=False,
                   num_devices=NCORES)
    praw_d = nc.dram_tensor("praw", [2, P, P], FP, kind="ExternalInput")
    sioff_d = nc.dram_tensor("sioff", [1, nblk], I32, kind="ExternalInput")
    bidx_d = nc.dram_tensor("bidx", [P, nslots], I32, kind="ExternalInput")
    out_d = nc.dram_tensor("out", [P, nslots * P], FP, kind="ExternalOutput")
    stat_d = nc.dram_tensor("stat", [(NB + 1) * P, P], FP)
    bdram_d = nc.dram_tensor("bdram", [NB * P, P], FP)

    with tile.TileContext(nc) as tc:
        with (
            tc.tile_pool(name="consts", bufs=1) as consts,
            tc.tile_pool(name="scratch", bufs=2) as scratch,
            tc.tile_pool(name="atab", bufs=1) as atab,
            tc.tile_pool(name="btab", bufs=1) as btabp,
            tc.tile_pool(name="sstage", bufs=2) as sstagep,
            tc.tile_pool(name="stage", bufs=int(os.environ.get("STAGE_BUFS", "8"))) as stagep,
            tc.tile_pool(name="mv16", bufs=int(os.environ.get("MV16_BUFS", "2"))) as mv16p,
            tc.tile_pool(name="mv", bufs=int(os.environ.get("MV_BUFS", "4"))) as mvp,
            tc.tile_pool(name="obuf", bufs=int(os.environ.get("OBUF_BUFS", "2"))) as obufp,
            tc.tile_pool(name="psum", bufs=int(os.environ.get("PSUM_BUFS", "8")), space="PSUM") as psump,
        ):
            ident = consts.tile([P, P], FP, tag="ident")
            make_identity(nc, ident[:])
            praw = consts.tile([P, 2, P], FP, tag="praw")
            nc.sync.dma_start(praw[:], praw_d[:].rearrange("p r c -> r p c"))
            sioff = consts.tile([1, nblk], I32, tag="sioff")
            bidx = consts.tile([P, nslots], I32, tag="bidx")
            nc.sync.dma_start(sioff[:], sioff_d[:])
            nc.sync.dma_start(bidx[:], bidx_d[:])

            # ---- phase A: primitives ----
            G, GT = _build_expm(nc, consts, psump, scratch, praw, ident)

            # ---- phase B: A2/A2T/A4/A4T doubling tables ----
            a2 = atab.tile([P, 4, P], FP, tag="a2")
            a2t = atab.tile([P, 4, P], FP, tag="a2t")
            for m in range(4):
                ps = psump.tile([P, 512], FP, tag="pos")
                _mm(nc, ps[:, :P], GT[m & 1][:], G[m >> 1][:])   # A2[m]
                nc.vector.tensor_copy(a2[:, m, :], ps[:, :P])
                ps2 = psump.tile([P, 512], FP, tag="pos")
                _mm(nc, ps2[:, :P], G[m >> 1][:], GT[m & 1][:])  # A2T[m]
                nc.vector.tensor_copy(a2t[:, m, :], ps2[:, :P])
            a4 = atab.tile([P, 16, P], FP, tag="a4")
            a4t = atab.tile([P, 16, P], FP, tag="a4t")
            a2f = a2[:].rearrange("r m c -> r (m c)")
            a2tf = a2t[:].rearrange("r m c -> r (m c)")
            for a in range(4):
                ps = psump.tile([P, 512], FP, tag="pos")
                _mm(nc, ps[:], a2t[:, a, :], a2f)        # A4[a+4b] over b
                for b2 in range(4):
                    nc.vector.tensor_copy(
                        a4[:, a + 4 * b2, :], ps[:, b2 * P : (b2 + 1) * P]
                    )
                # A4T[m] = A2T[m>>2] @ A2T[m&3]; fix g=m>>2: m = 4g+b contiguous
                ps2 = psump.tile([P, 512], FP, tag="pos")
                _mm(nc, ps2[:], a2[:, a, :], a2tf)
                nc.vector.tensor_copy(
                    a4t[:, 4 * a : 4 * a + 4, :].rearrange("r m c -> r (m c)"),
                    ps2[:],
                )

            # ---- phase C: S^T table (A8^T) -> stat_d[0:256], identity -> [256]
            a4tf = a4t[:].rearrange("r m c -> r (m c)")
            stat_v = stat_d[:].rearrange("(e r) c -> r e c", r=P)
            for g in range(16):
                for q in range(4):
                    sst = sstagep.tile([P, 4, P], FP, tag="sst")
                    ps = psump.tile([P, 512], FP, tag="pos")
                    # S^T[16g + (4q+j)] = A4T[g] @ A4T[4q+j], j=0..3
                    _mm(nc, ps[:], a4[:, g, :], a4tf[:, q * 512 : (q + 1) * 512])
                    nc.vector.tensor_copy(
                        sst[:].rearrange("r m c -> r (m c)"), ps[:]
                    )
                    nc.sync.dma_start(
                        stat_v[:, 16 * g + 4 * q : 16 * g + 4 * q + 4, :],
                        sst[:],
                    )
            nc.sync.dma_start(stat_v[:, NB : NB + 1, :], ident[:, None, :])

            # ---- phase D: B table (SBUF, optionally DRAM copy) ----
            btab = btabp.tile([P, NB, P], FP, tag="btab")
            nc.vector.tensor_copy(btab[:, 0, :], ident[:])
            nc.vector.tensor_copy(btab[:, 1, :], ident[:])
            for lvl in range(1, 8):
                p0, p1 = 1 << (lvl - 1), 1 << lvl
                for b in range(2):
                    for c0 in range(p0, p1, 4):
                        npar = min(4, p1 - c0)
                        ps = psump.tile([P, 512], FP, tag="pos")
                        _mm(
                            nc,
                            ps[:, : npar * P],
                            GT[b][:],
                            btab[:, c0 : c0 + npar, :].rearrange(
                                "r m c -> r (m c)"
                            ),
                        )
                        for j in range(npar):
                            nc.vector.tensor_copy(
                                btab[:, 2 * (c0 + j) + b, :],
                                ps[:, j * P : (j + 1) * P],
                            )
            nc.sync.dma_start(
                bdram_d[:].rearrange("(e r) c -> r e c", r=P), btab[:]
            )

            # ---- phase E: position loop ----
            # B16 blocks: 1 stationary stage (SWDGE dyn DMA) + 1 indirect
            # gather of 16 moving entries + 4 matmuls N=512 + 4 evacs.
            # B4 blocks: same with 4 entries / 1 matmul / 1 evac.
            with (
                nc.gpsimd.register("rg") as rg,
                nc.scalar.register("ra") as ra,
                nc.sync.register("rs") as rs,
            ):
                def do_block(blk, s0, size, ob, obase):
                    st = stagep.tile([P, P], FP, tag="st")
                    nc.sync.reg_load(rs, sioff[0:1, blk : blk + 1])
                    so = nc.sync.snap(rs)
                    nc.sync.dma_start(st[:], stat_d[bass.ds(so, P), :])
                    if size == 16:
                        mv = mv16p.tile([P, 16, P], FP, tag="mv16")
                    else:
                        mv = mvp.tile([P, 4, P], FP, tag="mv4")
                    for j in range(size):
                        if j % 2 == 0:
                            eng, reg = nc.gpsimd, rg
                        else:
                            eng, reg = nc.scalar, ra
                        eng.reg_load(reg, bidx[0:1, s0 + j : s0 + j + 1])
                        bo = eng.snap(reg)
                        eng.dma_start(mv[:, j, :], bdram_d[bass.ds(bo, P), :])
                    for q in range(size // 4):
                        ps = psump.tile([P, 512], FP, tag="pos")
                        _mm(
                            nc,
                            ps[:],
                            st[:],
                            mv[:, 4 * q : 4 * q + 4, :].rearrange(
                                "r m c -> r (m c)"
                            ),
                        )
                        nc.vector.tensor_copy(
                            ob[:, obase + 4 * q * P : obase + (4 * q + 4) * P],
                            ps[:],
                        )

                for b in range(n16):
                    ob = obufp.tile([P, 16 * P], FP, tag="ob")
                    do_block(b, b * 16, 16, ob, 0)
                    nc.sync.dma_start(
                        out_d[:, b * 16 * P : (b + 1) * 16 * P], ob[:]
                    )
                base16 = n16 * 16
                for c0 in range(0, n4, 4):
                    nbi = min(4, n4 - c0)
                    ob = obufp.tile([P, 16 * P], FP, tag="ob")
                    for k in range(nbi):
                        blk = n16 + c0 + k
                        do_block(blk, base16 + (c0 + k) * 4, 4, ob, k * 4 * P)
                    nc.sync.dma_start(
                        out_d[
                            :,
                            (base16 + c0 * 4) * P : (base16 + (c0 + nbi) * 4) * P,
                        ],
                        ob[:, : nbi * 4 * P],
                    )
    nc.compile()
    return nc


def _plan_blocks(unique):
    """Pack positions into 16-blocks and 4-blocks sharing a stationary entry.

    Returns (blocks16, blocks4) where each block is (ent, [bents...]) with
    bents padded with -1 markers replaced by 0 later, plus member position
    indices for slot mapping.
    """
    n = unique.shape[0]
    lo = unique & 255
    hi = unique >> 8
    ent = np.where(hi > 0, lo, IDENT_ENTRY)
    bent = np.where(hi > 0, hi, unique)  # hi==0 -> out = I @ B[pos]
    order = np.argsort(ent, kind="stable")
    es = ent[order]
    bounds = np.flatnonzero(np.r_[True, es[1:] != es[:-1], True])

    blocks16, blocks4 = [], []
    for s, e in zip(bounds[:-1], bounds[1:]):
        idxs = order[s:e]
        v = int(es[s])
        g = len(idxs)
        q0 = 0
        while g - q0 >= 16:
            blocks16.append((v, idxs[q0 : q0 + 16]))
            q0 += 16
        while q0 < g:
            blocks4.append((v, idxs[q0 : q0 + 4]))
            q0 += 4
    return blocks16, blocks4, bent


def kernel(unique, primitives_raw, identity=None, **_):
    unique = np.asarray(unique)
    praw = np.ascontiguousarray(np.asarray(primitives_raw, np.float32))

    blocks16, blocks4, bent = _plan_blocks(unique.astype(np.int64))
    n16 = -(-len(blocks16) // NCORES)
    n4 = -(-len(blocks4) // NCORES)
    while len(blocks16) < NCORES * n16:
        blocks16.append((IDENT_ENTRY, np.empty(0, np.int64)))
    while len(blocks4) < NCORES * n4:
        blocks4.append((IDENT_ENTRY, np.empty(0, np.int64)))
    nslots = n16 * 16 + n4 * 4

    # per-core inputs + slot mapping
    slot_of_pos = np.zeros(unique.shape[0], np.int64)
    sioff = np.zeros((NCORES, n16 + n4), np.int32)
    bidx = np.zeros((NCORES, P, nslots), np.int32)
    rows = np.arange(P, dtype=np.int32)
    for i, (v, mem) in enumerate(blocks16):
        c, k = divmod(i, n16)
        sioff[c, k] = v * P
        for j, pidx in enumerate(mem):
            bidx[c, :, k * 16 + j] = int(bent[pidx]) * P + rows
            slot_of_pos[pidx] = c * nslots + k * 16 + j
        for j in range(len(mem), 16):
            bidx[c, :, k * 16 + j] = rows
    for i, (v, mem) in enumerate(blocks4):
        c, k = divmod(i, n4)
        sioff[c, n16 + k] = v * P
        base = n16 * 16 + k * 4
        for j, pidx in enumerate(mem):
            bidx[c, :, base + j] = int(bent[pidx]) * P + rows
            slot_of_pos[pidx] = c * nslots + base + j
        for j in range(len(mem), 4):
            bidx[c, :, base + j] = rows

    key = (n16, n4)
    if key not in _prog_cache:
        _prog_cache[key] = build_program(n16, n4)
    nc = _prog_cache[key]

    in_maps = [
        {
            "praw": praw,
            "sioff": np.ascontiguousarray(sioff[c].reshape(1, -1)),
            "bidx": np.ascontiguousarray(bidx[c]),
        }
        for c in range(NCORES)
    ]
    global _last_ctx
    _last_ctx = (nc, in_maps)
    res = run_bass_kernel_spmd(nc, in_maps, list(range(NCORES)))
    outs = np.concatenate(
        [
            res.results[c]["out"]
            .reshape(P, nslots, P)
            .transpose(1, 0, 2)
            for c in range(NCORES)
        ],
        axis=0,
    )
    return np.ascontiguousarray(outs[slot_of_pos]).astype(np.float32)


if __name__ == "__main__":
    rng = np.random.default_rng(0)
    u = rng.integers(1, 65536, 64).astype(np.int32)
    pr = rng.random((2, P, P), np.float32)
    o = kernel(u, pr)
    print(o.shape, o.dtype)



# revision 3
# speedup vs baseline: 3.9968x; 3.9968x over previous
"""Trainium2 Bass kernel for nn_BinaryPathEncoder.

Math: out[n] = prod_k G_{b_k(pos_n)} over the binary digits of pos_n below
its leading 1 (LSB first, leftmost), where G_b = expm(-(P_b - P_b^T)).

Decomposition (pos = hi*256 + lo):
  hi >= 1: out = A8(lo) @ B[hi], A8(lo) = all-valid 8-bit product,
           B[h] = M(h) for h < 256.
  hi == 0: out = I @ B[pos].

Device program (ONE SPMD program on 8 cores, specialized per input):
  - fp32 Taylor scaling-squaring expm -> G, G^T
  - fp32r (full-rate, ~1.6e-4) products build A2/A4/B4 tables, then the
    256-entry B table and the 256-entry A8^T stationary table, both
    rounded once to fp16 and kept entirely in SBUF.
  - core id (int32 input) selects one of 8 statically-generated branches
    (nested tc.If/Else); each branch is that core's position loop with
    static table slices: per slot one N=128 fp16 matmul
    psum_quadrant = stat[lo]^T.T @ btab[hi], DVE/ACT-alternated PSUM
    evacuation to fp16, and 2MB output DMAs. No dynamic addressing at all.
Host: dedups positions, sorts by stationary key, splits contiguously
across cores, scatters results back, converts fp16 -> fp32.
"""

import hashlib

import numpy as np
import ml_dtypes

import concourse.bass as bass
import concourse.bacc as bacc
import concourse.mybir as mybir
import concourse.tile as tile
import concourse.tile_utils as tile_utils
tile_utils.max_sbuf_usage = 206 * 1024
from concourse.bass_utils import run_bass_kernel_spmd
from concourse.masks import make_identity

FP = mybir.dt.float32
FR = mybir.dt.float32r
F16 = mybir.dt.float16
I32 = mybir.dt.int32
P = 128
NCORES = 8
S_EXP = 5          # scaling-squaring: X = -H / 2^S_EXP
ORDER = 18         # Taylor order
IDENT_KEY = 256    # stationary key for hi==0 positions
WS = 64            # output wave size in slots

_prog_cache = {}
_last_ctx = None


def _mm(nc, out, lhsT, rhs):
    nc.tensor.matmul(out, lhsT=lhsT, rhs=rhs, start=True, stop=True)


def _build_expm(nc, consts, psump, scratch, praw, ident, g, gt):
    """g[:, b, :] = expm(-H_b) (fp32r), gt[:, b, :] = its transpose."""
    for b in range(2):
        pb = praw[:, b, :]
        ps_t = psump.tile([P, 512], FP, tag="pos")
        nc.tensor.transpose(out=ps_t[:, :P], in_=pb, identity=ident[:])
        xt = scratch.tile([P, P], FP, tag=f"xt{b}", bufs=1)
        # XT = (P - P^T)/2^s; lhsT=XT gives out = (-H/2^s) @ rhs since H^T=-H
        nc.vector.tensor_tensor(
            out=xt[:], in0=pb, in1=ps_t[:, :P], op=mybir.AluOpType.subtract
        )
        nc.vector.tensor_scalar_mul(xt[:], xt[:], 1.0 / (1 << S_EXP))

        t_cur = scratch.tile([P, P], FP, tag="tay")
        nc.vector.tensor_copy(t_cur[:], ident[:])
        for k in range(ORDER, 0, -1):
            ps = psump.tile([P, 512], FP, tag="pos")
            _mm(nc, ps[:, :P], xt[:], t_cur[:])
            t_nxt = scratch.tile([P, P], FP, tag="tay")
            nc.vector.tensor_scalar_mul(t_nxt[:], ps[:, :P], 1.0 / k)
            nc.vector.tensor_add(t_nxt[:], t_nxt[:], ident[:])
            t_cur = t_nxt
        # U = T^T
        ps_u = psump.tile([P, 512], FP, tag="pos")
        nc.tensor.transpose(out=ps_u[:, :P], in_=t_cur[:], identity=ident[:])
        u_cur = scratch.tile([P, P], FP, tag="tayu")
        nc.vector.tensor_copy(u_cur[:], ps_u[:, :P])
        for _ in range(S_EXP):
            ps1 = psump.tile([P, 512], FP, tag="pos")
            ps2 = psump.tile([P, 512], FP, tag="pos")
            _mm(nc, ps1[:, :P], u_cur[:], t_cur[:])   # T' = T @ T
            _mm(nc, ps2[:, :P], t_cur[:], u_cur[:])   # U' = (T@T)^T
            t_cur = scratch.tile([P, P], FP, tag="tay")
            u_cur = scratch.tile([P, P], FP, tag="tayu")
            nc.vector.tensor_copy(t_cur[:], ps1[:, :P])
            nc.vector.tensor_copy(u_cur[:], ps2[:, :P])
        nc.vector.tensor_copy(g[:, b, :], t_cur[:])
        nc.vector.tensor_copy(gt[:, b, :], u_cur[:])


def build_program(core_slots, nslots):
    nc = bacc.Bacc("TRN2", target_bir_lowering=False, debug=False,
                   num_devices=NCORES)
    praw_d = nc.dram_tensor("praw", [2, P, P], FP, kind="ExternalInput")
    cid_d = nc.dram_tensor("cid", [1, 1], I32, kind="ExternalInput")
    out_d = nc.dram_tensor("out", [P, nslots * P], F16, kind="ExternalOutput")

    with tile.TileContext(nc) as tc:
        with (
            tc.tile_pool(name="consts", bufs=1) as consts,
            tc.tile_pool(name="scratch", bufs=2) as scratch,
            tc.tile_pool(name="obuf", bufs=2) as obufp,
            tc.tile_pool(name="psum", bufs=8, space="PSUM") as psump,
        ):
            ident = consts.tile([P, P], FP, tag="ident")
            make_identity(nc, ident[:])
            identh = consts.tile([P, P], F16, tag="identh")
            nc.vector.tensor_copy(identh[:], ident[:])
            identr = consts.tile([P, P], FR, tag="identr")
            nc.vector.tensor_copy(identr[:], ident[:])
            praw = consts.tile([P, 2, P], FP, tag="praw")
            nc.sync.dma_start(praw[:], praw_d[:].rearrange("p r c -> r p c"))
            cidt = consts.tile([1, 1], I32, tag="cid")
            nc.sync.dma_start(cidt[:], cid_d[:])

            g = consts.tile([P, 2, P], FR, tag="g")
            gt = consts.tile([P, 2, P], FR, tag="gt")
            a2 = consts.tile([P, 4, P], FR, tag="a2")
            a2t = consts.tile([P, 4, P], FR, tag="a2t")
            a4 = consts.tile([P, 16, P], FR, tag="a4")
            a4t = consts.tile([P, 16, P], FR, tag="a4t")
            b4 = consts.tile([P, 16, P], FR, tag="b4")
            btab = consts.tile([P, 256, P], F16, tag="btab")
            stab = consts.tile([P, 256, P], F16, tag="stab")

            # ---- phase A: primitives (fp32 Taylor) ----
            _build_expm(nc, consts, psump, scratch, praw, ident, g, gt)

            gf = g[:].rearrange("r m c -> r (m c)")
            gtf = gt[:].rearrange("r m c -> r (m c)")

            # ---- phase B: A2/A2T/A4/A4T (fp32r, N>=256 full rate) ----
            for a in range(2):
                ps = psump.tile([P, 512], FP, tag="pos")
                _mm(nc, ps[:, :256], gt[:, a, :], gf)      # [A2[a], A2[a+2]]
                nc.vector.tensor_copy(
                    a2[:, a : a + 3 : 2, :],
                    ps[:, :256].rearrange("r (m c) -> r m c", c=P),
                )
            for b in range(2):
                ps = psump.tile([P, 512], FP, tag="pos")
                _mm(nc, ps[:, :256], g[:, b, :], gtf)      # [A2T[2b], A2T[2b+1]]
                nc.vector.tensor_copy(
                    a2t[:, 2 * b : 2 * b + 2, :],
                    ps[:, :256].rearrange("r (m c) -> r m c", c=P),
                )
            a2f = a2[:].rearrange("r m c -> r (m c)")
            a2tf = a2t[:].rearrange("r m c -> r (m c)")
            for a in range(4):
                ps = psump.tile([P, 512], FP, tag="pos")
                _mm(nc, ps[:], a2t[:, a, :], a2f)          # A4[a+4g] over g
                nc.vector.tensor_copy(
                    a4[:, a : a + 13 : 4, :],
                    ps[:].rearrange("r (m c) -> r m c", c=P),
                )
            for gg in range(4):
                ps = psump.tile([P, 512], FP, tag="pos")
                _mm(nc, ps[:], a2[:, gg, :], a2tf)         # A4T[4g+j] over j
                nc.vector.tensor_copy(
                    a4t[:, 4 * gg : 4 * gg + 4, :],
                    ps[:].rearrange("r (m c) -> r m c", c=P),
                )

            # ---- phase C: B4 table (M(h), h<16) ----
            nc.vector.tensor_copy(b4[:, 0, :], identr[:])
            nc.vector.tensor_copy(b4[:, 1, :], identr[:])
            nc.vector.tensor_copy(b4[:, 2, :], g[:, 0, :])
            nc.vector.tensor_copy(b4[:, 3, :], g[:, 1, :])
            for b in range(2):
                ps = psump.tile([P, 512], FP, tag="pos")
                _mm(nc, ps[:, :256], gt[:, b, :],
                    b4[:, 2:4, :].rearrange("r m c -> r (m c)"))
                # [B4[4+b], B4[6+b]]
                nc.vector.tensor_copy(
                    b4[:, 4 + b : 8 : 2, :],
                    ps[:, :256].rearrange("r (m c) -> r m c", c=P),
                )
            for b in range(2):
                ps = psump.tile([P, 512], FP, tag="pos")
                _mm(nc, ps[:], gt[:, b, :],
                    b4[:, 4:8, :].rearrange("r m c -> r (m c)"))
                # [B4[8+b], B4[10+b], B4[12+b], B4[14+b]]
                nc.vector.tensor_copy(
                    b4[:, 8 + b : 16 : 2, :],
                    ps[:].rearrange("r (m c) -> r m c", c=P),
                )

            # ---- phase D: B table fp16 (256 entries, SBUF) ----
            b4f = b4[:].rearrange("r m c -> r (m c)")
            nc.vector.tensor_copy(
                btab[:, 0:16, :].rearrange("r m c -> r (m c)"), b4f
            )
            for m in range(16):
                for q in range(4):
                    ps = psump.tile([P, 512], FP, tag="pos")
                    _mm(nc, ps[:], a4t[:, m, :], b4f[:, q * 512 : (q + 1) * 512])
                    # entries m+16c, c=4q..4q+3 (skip c==0: h<16 handled above)
                    if q == 0:
                        nc.vector.tensor_copy(
                            btab[:, m + 16 : m + 49 : 16, :],
                            ps[:, 128:512].rearrange("r (m c) -> r m c", c=P),
                        )
                    else:
                        c0 = m + 16 * 4 * q
                        nc.vector.tensor_copy(
                            btab[:, c0 : c0 + 49 : 16, :],
                            ps[:].rearrange("r (m c) -> r m c", c=P),
                        )

            # ---- phase E: stationary table A8T fp16 (256 entries, SBUF) ----
            a4tf = a4t[:].rearrange("r m c -> r (m c)")
            for gg in range(16):
                for q in range(4):
                    ps = psump.tile([P, 512], FP, tag="pos")
                    _mm(nc, ps[:], a4[:, gg, :], a4tf[:, q * 512 : (q + 1) * 512])
                    nc.vector.tensor_copy(
                        stab[:, 16 * gg + 4 * q : 16 * gg + 4 * q + 4, :],
                        ps[:].rearrange("r (m c) -> r m c", c=P),
                    )

            # ---- phase F: per-core position loops (static, branch on cid) ----
            cid = nc.values_load(cidt[0:1, 0:1], min_val=0, max_val=NCORES - 1,
                                 skip_runtime_bounds_check=True)
            copy_f = mybir.ActivationFunctionType.Copy

            def body(c):
                slots = core_slots[c]
                nq = nslots // 4
                for w0 in range(0, nq, WS // 4):
                    wq = min(WS // 4, nq - w0)
                    ob = obufp.tile([P, WS * P], F16, tag="ob")
                    for qi in range(wq):
                        q = w0 + qi
                        ps = psump.tile([P, 512], FP, tag="pos")
                        for j in range(4):
                            key, bent = slots[4 * q + j]
                            lhsT = identh[:] if key == IDENT_KEY else stab[:, key, :]
                            nc.tensor.matmul(
                                ps[:, j * P : (j + 1) * P],
                                lhsT=lhsT,
                                rhs=btab[:, bent, :],
                                start=True,
                                stop=True,
                            )
                        dst = ob[:, qi * 512 : (qi + 1) * 512]
                        if qi % 2 == 0:
                            nc.vector.tensor_copy(dst, ps[:])
                        else:
                            nc.scalar.activation(dst, ps[:], copy_f)
                    nc.sync.dma_start(
                        out_d[:, w0 * 4 * P : (w0 * 4 + wq * 4) * P],
                        ob[:, : wq * 4 * P],
                    )

            def emit(c):
                if c == NCORES - 1:
                    body(c)
                    return
                with tc.If(cid == c) as cmp:
                    body(c)
                with cmp.Else():
                    emit(c + 1)

            emit(0)
    nc.compile()
    return nc


def _plan(unique):
    """Dedup + per-core static slot lists sorted by stationary key."""
    uq = np.asarray(unique).astype(np.int64)
    dvals, dinv = np.unique(uq, return_inverse=True)
    lo = dvals & 255
    hi = dvals >> 8
    key = np.where(hi > 0, lo, IDENT_KEY)
    bent = np.where(hi > 0, hi, dvals)
    order = np.argsort(key, kind="stable")
    n = len(dvals)
    bounds = [(c * n) // NCORES for c in range(NCORES + 1)]
    nslots = -(-max(bounds[c + 1] - bounds[c] for c in range(NCORES)) // 4) * 4
    core_slots = []
    gslot = np.zeros(n, np.int64)
    for c in range(NCORES):
        idxs = order[bounds[c] : bounds[c + 1]]
        sl = [(int(key[i]), int(bent[i])) for i in idxs]
        gslot[idxs] = c * nslots + np.arange(len(sl))
        sl += [(IDENT_KEY, 0)] * (nslots - len(sl))
        core_slots.append(sl)
    return core_slots, nslots, gslot, dinv


def kernel(unique, primitives_raw, identity=None, **_):
    unique = np.asarray(unique)
    praw = np.ascontiguousarray(np.asarray(primitives_raw, np.float32))

    ckey = hashlib.sha256(unique.tobytes()).hexdigest()
    if ckey not in _prog_cache:
        core_slots, nslots, gslot, dinv = _plan(unique)
        nc = build_program(core_slots, nslots)
        _prog_cache.clear()
        _prog_cache[ckey] = (nc, nslots, gslot, dinv)
    nc, nslots, gslot, dinv = _prog_cache[ckey]

    in_maps = [
        {"praw": praw, "cid": np.array([[c]], np.int32)}
        for c in range(NCORES)
    ]
    global _last_ctx
    _last_ctx = (nc, in_maps)
    res = run_bass_kernel_spmd(nc, in_maps, list(range(NCORES)))
    outs = np.concatenate(
        [
            np.asarray(res.results[c]["out"])
            .reshape(P, nslots, P)
            .transpose(1, 0, 2)
            for c in range(NCORES)
        ],
        axis=0,
    )
    return outs[gslot[dinv]].astype(np.float32)


if __name__ == "__main__":
    rng = np.random.default_rng(0)
    u = rng.integers(1, 65536, 256).astype(np.int32)
    pr = rng.random((2, P, P), np.float32)
    o = kernel(u, pr)
    print(o.shape, o.dtype)


# revision 6
# speedup vs baseline: 5.1917x; 1.2989x over previous
"""Trainium2 Bass kernel for nn_BinaryPathEncoder.

Math: out[n] = prod_k G_{b_k(pos_n)} over the binary digits of pos_n below
its leading 1 (LSB first, leftmost), where G_b = expm(-(P_b - P_b^T)).

Decomposition (pos = hi*256 + lo):
  hi >= 1: out = A8(lo) @ B[hi], A8(lo) = all-valid 8-bit product,
           B[h] = M(h) for h < 256.
  hi == 0: out = I @ B[pos].

Device program (ONE SPMD program on 8 cores, specialized per input):
  - fp32 Taylor scaling-squaring expm -> G, G^T (both chains interleaved,
    elementwise work split DVE / GpSimd)
  - fp32r (full-rate, ~1.6e-4) products build A2/A4/B4 tables, then the
    256-entry A8^T stationary table and the 256-entry B table, rounded
    once to fp16 and kept entirely in SBUF; PSUM evacuations cycle
    DVE/ACT/GpSimd.
  - core id (int32 input) selects one of 8 statically-generated branches
    (nested tc.If/Else); each branch is that core's position loop with
    static table slices: per slot one N=128 fp16 matmul
    psum_quadrant = stat[lo]^T.T @ btab[hi], three-engine-cycled PSUM
    evacuation to fp16, and 2MB output DMAs. No dynamic addressing at all.
Host: dedups positions, sorts by stationary key, splits contiguously
across cores, scatters results back, converts fp16 -> fp32.
"""

import hashlib

import numpy as np
import ml_dtypes

import concourse.bass as bass
import concourse.bacc as bacc
import concourse.mybir as mybir
import concourse.tile as tile
import concourse.tile_utils as tile_utils
tile_utils.max_sbuf_usage = 206 * 1024
from concourse.bass_utils import run_bass_kernel_spmd
from concourse.masks import make_identity

FP = mybir.dt.float32
FR = mybir.dt.float32r
F16 = mybir.dt.float16
I32 = mybir.dt.int32
P = 128
NCORES = 8
S_EXP = 5          # scaling-squaring: X = -H / 2^S_EXP
ORDER = 18         # Taylor order
IDENT_KEY = 256    # stationary key for hi==0 positions
WS = 64            # output wave size in slots

_prog_cache = {}
_last_ctx = None


def _mm(nc, out, lhsT, rhs):
    nc.tensor.matmul(out, lhsT=lhsT, rhs=rhs, start=True, stop=True)


def _build_expm(nc, psump, scratch, praw, ident, g, gt):
    """g[:, b, :] = expm(-H_b) (fp32r), gt[:, b, :] = its transpose.

    The two chains (b=0,1) are interleaved. PSUM-reading elementwise work
    splits DVE (b=0) / ACT (b=1); GpSimd (no PSUM access) takes chain 1's
    SBUF-side adds so no single engine serializes the build.
    """
    copy_f = mybir.ActivationFunctionType.Copy

    def ps_scale(b, dst, src, sc):      # dst = src * sc (src in PSUM)
        if b == 0:
            nc.vector.tensor_scalar_mul(dst, src, sc)
        else:
            nc.scalar.activation(dst, src, copy_f, scale=sc)

    def sb_add(b, dst, a, c):           # dst = a + c (all SBUF)
        (nc.vector if b == 0 else nc.gpsimd).tensor_add(dst, a, c)

    def ps_copy(b, dst, src):           # dst = src (src in PSUM)
        if b == 0:
            nc.vector.tensor_copy(dst, src)
        else:
            nc.scalar.activation(dst, src, copy_f)

    def sb_copy(b, dst, src):           # dst = src (all SBUF)
        (nc.vector if b == 0 else nc.gpsimd).tensor_copy(dst, src)

    xt, t_cur, u_cur = [None, None], [None, None], [None, None]
    for b in range(2):
        pb = praw[:, b, :]
        ps_t = psump.tile([P, 512], FP, tag="pos")
        nc.tensor.transpose(out=ps_t[:, :P], in_=pb, identity=ident[:])
        x = scratch.tile([P, P], FP, tag=f"xt{b}", bufs=1)
        # XT = (P - P^T)/2^s; lhsT=XT gives out = (-H/2^s) @ rhs since H^T=-H
        nc.vector.tensor_tensor(
            out=x[:], in0=pb, in1=ps_t[:, :P], op=mybir.AluOpType.subtract
        )
        nc.vector.tensor_scalar_mul(x[:], x[:], 1.0 / (1 << S_EXP))
        xt[b] = x
        t = scratch.tile([P, P], FP, tag=f"tay{b}")
        sb_copy(b, t[:], ident[:])
        t_cur[b] = t
    for k in range(ORDER, 0, -1):
        for b in range(2):
            ps = psump.tile([P, 512], FP, tag="pos")
            _mm(nc, ps[:, :P], xt[b][:], t_cur[b][:])
            t_nxt = scratch.tile([P, P], FP, tag=f"tay{b}")
            ps_scale(b, t_nxt[:], ps[:, :P], 1.0 / k)
            sb_add(b, t_nxt[:], t_nxt[:], ident[:])
            t_cur[b] = t_nxt
    for b in range(2):
        ps_u = psump.tile([P, 512], FP, tag="pos")
        nc.tensor.transpose(out=ps_u[:, :P], in_=t_cur[b][:], identity=ident[:])
        u = scratch.tile([P, P], FP, tag=f"tayu{b}")
        ps_copy(b, u[:], ps_u[:, :P])
        u_cur[b] = u
    for _ in range(S_EXP):
        for b in range(2):
            ps1 = psump.tile([P, 512], FP, tag="pos")
            ps2 = psump.tile([P, 512], FP, tag="pos")
            _mm(nc, ps1[:, :P], u_cur[b][:], t_cur[b][:])   # T' = T @ T
            _mm(nc, ps2[:, :P], t_cur[b][:], u_cur[b][:])   # U' = (T@T)^T
            t_cur[b] = scratch.tile([P, P], FP, tag=f"tay{b}", name=f"tay{b}")
            u_cur[b] = scratch.tile([P, P], FP, tag=f"tayu{b}", name=f"tayu{b}")
            ps_copy(b, t_cur[b][:], ps1[:, :P])
            ps_copy(b, u_cur[b][:], ps2[:, :P])
    for b in range(2):
        ps_copy(b, g[:, b, :], t_cur[b][:])
        ps_copy(b, gt[:, b, :], u_cur[b][:])


def build_program(core_slots, nslots):
    nc = bacc.Bacc("TRN2", target_bir_lowering=False, debug=False,
                   num_devices=NCORES)
    praw_d = nc.dram_tensor("praw", [2, P, P], FP, kind="ExternalInput")
    cid_d = nc.dram_tensor("cid", [1, 1], I32, kind="ExternalInput")
    out_d = nc.dram_tensor("out", [P, nslots * P], F16, kind="ExternalOutput")

    copy_f = mybir.ActivationFunctionType.Copy
    with tile.TileContext(nc) as tc:
        with (
            tc.tile_pool(name="consts", bufs=1) as consts,
            tc.tile_pool(name="scratch", bufs=2) as scratch,
            tc.tile_pool(name="obuf", bufs=2) as obufp,
            tc.tile_pool(name="psum", bufs=8, space="PSUM") as psump,
        ):
            ident = consts.tile([P, P], FP, tag="ident")
            make_identity(nc, ident[:])
            identh = consts.tile([P, P], F16, tag="identh")
            nc.vector.tensor_copy(identh[:], ident[:])
            identr = consts.tile([P, P], FR, tag="identr")
            nc.vector.tensor_copy(identr[:], ident[:])
            praw = consts.tile([P, 2, P], FP, tag="praw")
            nc.sync.dma_start(praw[:], praw_d[:].rearrange("p r c -> r p c"))
            cidt = consts.tile([1, 1], I32, tag="cid")
            nc.sync.dma_start(cidt[:], cid_d[:])
            cid = nc.values_load(cidt[0:1, 0:1], min_val=0, max_val=NCORES - 1,
                                 skip_runtime_bounds_check=True)

            g = consts.tile([P, 2, P], FR, tag="g")
            gt = consts.tile([P, 2, P], FR, tag="gt")
            a2 = consts.tile([P, 4, P], FR, tag="a2")
            a2t = consts.tile([P, 4, P], FR, tag="a2t")
            a4 = consts.tile([P, 16, P], FR, tag="a4")
            a4t = consts.tile([P, 16, P], FR, tag="a4t")
            b4 = consts.tile([P, 16, P], FR, tag="b4")
            btab = consts.tile([P, 256, P], F16, tag="btab")
            stab = consts.tile([P, 256, P], F16, tag="stab")

            # fp16 PSUM evacuations alternate DVE / ACT (GpSimd cannot
            # access PSUM); fp32r ones stay on DVE
            def evac16(i, dst, src):
                if i % 2 == 0:
                    nc.vector.tensor_copy(dst, src)
                else:
                    nc.scalar.activation(dst, src, copy_f)

            # ---- phase A: primitives (fp32 Taylor) ----
            _build_expm(nc, psump, scratch, praw, ident, g, gt)

            gf = g[:].rearrange("r m c -> r (m c)")
            gtf = gt[:].rearrange("r m c -> r (m c)")

            # ---- phase B: A2/A2T/A4/A4T (fp32r, N>=256 full rate) ----
            for a in range(2):
                ps = psump.tile([P, 512], FP, tag="pos")
                _mm(nc, ps[:, :256], gt[:, a, :], gf)      # [A2[a], A2[a+2]]
                nc.vector.tensor_copy(
                    a2[:, a : a + 3 : 2, :],
                    ps[:, :256].rearrange("r (m c) -> r m c", c=P),
                )
            for b in range(2):
                ps = psump.tile([P, 512], FP, tag="pos")
                _mm(nc, ps[:, :256], g[:, b, :], gtf)      # [A2T[2b], A2T[2b+1]]
                nc.vector.tensor_copy(
                    a2t[:, 2 * b : 2 * b + 2, :],
                    ps[:, :256].rearrange("r (m c) -> r m c", c=P),
                )
            a2f = a2[:].rearrange("r m c -> r (m c)")
            a2tf = a2t[:].rearrange("r m c -> r (m c)")
            for a in range(4):
                ps = psump.tile([P, 512], FP, tag="pos")
                _mm(nc, ps[:], a2t[:, a, :], a2f)          # A4[a+4g] over g
                nc.vector.tensor_copy(
                    a4[:, a : a + 13 : 4, :],
                    ps[:].rearrange("r (m c) -> r m c", c=P),
                )
            for gg in range(4):
                ps = psump.tile([P, 512], FP, tag="pos")
                _mm(nc, ps[:], a2[:, gg, :], a2tf)         # A4T[4g+j] over j
                nc.vector.tensor_copy(
                    a4t[:, 4 * gg : 4 * gg + 4, :],
                    ps[:].rearrange("r (m c) -> r m c", c=P),
                )

            # ---- phase C: B4 table (M(h), h<16) ----
            nc.vector.tensor_copy(b4[:, 0, :], identr[:])
            nc.vector.tensor_copy(b4[:, 1, :], identr[:])
            nc.vector.tensor_copy(b4[:, 2, :], g[:, 0, :])
            nc.vector.tensor_copy(b4[:, 3, :], g[:, 1, :])
            for b in range(2):
                ps = psump.tile([P, 512], FP, tag="pos")
                _mm(nc, ps[:, :256], gt[:, b, :],
                    b4[:, 2:4, :].rearrange("r m c -> r (m c)"))
                # [B4[4+b], B4[6+b]]
                nc.vector.tensor_copy(
                    b4[:, 4 + b : 8 : 2, :],
                    ps[:, :256].rearrange("r (m c) -> r m c", c=P),
                )
            for b in range(2):
                ps = psump.tile([P, 512], FP, tag="pos")
                _mm(nc, ps[:], gt[:, b, :],
                    b4[:, 4:8, :].rearrange("r m c -> r (m c)"))
                # [B4[8+b], B4[10+b], B4[12+b], B4[14+b]]
                nc.vector.tensor_copy(
                    b4[:, 8 + b : 16 : 2, :],
                    ps[:].rearrange("r (m c) -> r m c", c=P),
                )

            # ---- phase D: stationary table A8T fp16 (256 entries, SBUF) ----
            # (before the B table: the position loops wait on btab's last
            #  write, so finish stab first)
            a4tf = a4t[:].rearrange("r m c -> r (m c)")
            ei = 0
            for gg in range(16):
                for q in range(4):
                    ps = psump.tile([P, 512], FP, tag="pos")
                    _mm(nc, ps[:], a4[:, gg, :], a4tf[:, q * 512 : (q + 1) * 512])
                    evac16(
                        ei,
                        stab[:, 16 * gg + 4 * q : 16 * gg + 4 * q + 4, :],
                        ps[:].rearrange("r (m c) -> r m c", c=P),
                    )
                    ei += 1

            # ---- phase E: B table fp16 (256 entries, SBUF) ----
            b4f = b4[:].rearrange("r m c -> r (m c)")
            nc.scalar.activation(
                btab[:, 0:16, :].rearrange("r m c -> r (m c)"), b4f, copy_f
            )
            for m in range(16):
                for q in range(4):
                    ps = psump.tile([P, 512], FP, tag="pos")
                    _mm(nc, ps[:], a4t[:, m, :], b4f[:, q * 512 : (q + 1) * 512])
                    # entries m+16c, c=4q..4q+3 (skip c==0: h<16 handled above)
                    if q == 0:
                        evac16(
                            ei,
                            btab[:, m + 16 : m + 49 : 16, :],
                            ps[:, 128:512].rearrange("r (m c) -> r m c", c=P),
                        )
                    else:
                        c0 = m + 64 * q
                        evac16(
                            ei,
                            btab[:, c0 : c0 + 49 : 16, :],
                            ps[:].rearrange("r (m c) -> r m c", c=P),
                        )
                    ei += 1

            # ---- phase F: per-core position loops (static, branch on cid) ----
            def body(c):
                slots = core_slots[c]
                nq = nslots // 4
                for w0 in range(0, nq, WS // 4):
                    wq = min(WS // 4, nq - w0)
                    ob = obufp.tile([P, WS * P], F16, tag="ob")
                    for qi in range(wq):
                        q = w0 + qi
                        ps = psump.tile([P, 512], FP, tag="pos")
                        for j in range(4):
                            key, bent = slots[4 * q + j]
                            lhsT = identh[:] if key == IDENT_KEY else stab[:, key, :]
                            nc.tensor.matmul(
                                ps[:, j * P : (j + 1) * P],
                                lhsT=lhsT,
                                rhs=btab[:, bent, :],
                                start=True,
                                stop=True,
                            )
                        evac16(qi, ob[:, qi * 512 : (qi + 1) * 512], ps[:])
                    nc.sync.dma_start(
                        out_d[:, w0 * 4 * P : (w0 * 4 + wq * 4) * P],
                        ob[:, : wq * 4 * P],
                    )

            def emit(c):
                if c == NCORES - 1:
                    body(c)
                    return
                with tc.If(cid == c) as cmp:
                    body(c)
                with cmp.Else():
                    emit(c + 1)

            emit(0)
    nc.compile()
    return nc


def _plan(unique):
    """Dedup + per-core static slot lists sorted by stationary key."""
    uq = np.asarray(unique).astype(np.int64)
    dvals, dinv = np.unique(uq, return_inverse=True)
    lo = dvals & 255
    hi = dvals >> 8
    key = np.where(hi > 0, lo, IDENT_KEY)
    bent = np.where(hi > 0, hi, dvals)
    order = np.argsort(key, kind="stable")
    n = len(dvals)
    bounds = [(c * n) // NCORES for c in range(NCORES + 1)]
    nslots = -(-max(bounds[c + 1] - bounds[c] for c in range(NCORES)) // 4) * 4
    core_slots = []
    gslot = np.zeros(n, np.int64)
    for c in range(NCORES):
        idxs = order[bounds[c] : bounds[c + 1]]
        sl = [(int(key[i]), int(bent[i])) for i in idxs]
        gslot[idxs] = c * nslots + np.arange(len(sl))
        sl += [(IDENT_KEY, 0)] * (nslots - len(sl))
        core_slots.append(sl)
    return core_slots, nslots, gslot, dinv


def kernel(unique, primitives_raw, identity=None, **_):
    unique = np.asarray(unique)
    praw = np.ascontiguousarray(np.asarray(primitives_raw, np.float32))

    ckey = hashlib.sha256(unique.tobytes()).hexdigest()
    if ckey not in _prog_cache:
        core_slots, nslots, gslot, dinv = _plan(unique)
        nc = build_program(core_slots, nslots)
        _prog_cache.clear()
        _prog_cache[ckey] = (nc, nslots, gslot, dinv)
    nc, nslots, gslot, dinv = _prog_cache[ckey]

    in_maps = [
        {"praw": praw, "cid": np.array([[c]], np.int32)}
        for c in range(NCORES)
    ]
    global _last_ctx
    _last_ctx = (nc, in_maps)
    res = run_bass_kernel_spmd(nc, in_maps, list(range(NCORES)))
    outs = np.concatenate(
        [
            np.asarray(res.results[c]["out"])
            .reshape(P, nslots, P)
            .transpose(1, 0, 2)
            for c in range(NCORES)
        ],
        axis=0,
    )
    return outs[gslot[dinv]].astype(np.float32)


if __name__ == "__main__":
    rng = np.random.default_rng(0)
    u = rng.integers(1, 65536, 256).astype(np.int32)
    pr = rng.random((2, P, P), np.float32)
    o = kernel(u, pr)
    print(o.shape, o.dtype)


# revision 7
# speedup vs baseline: 6.2365x; 1.2013x over previous
"""Trainium2 Bass kernel for nn_BinaryPathEncoder.

Math: out[n] = prod_k G_{b_k(pos_n)} over the binary digits of pos_n below
its leading 1 (LSB first, leftmost), where G_b = expm(-(P_b - P_b^T)).

Decomposition (pos = hi*256 + lo):
  hi >= 1: out = A8(lo) @ B[hi], A8(lo) = all-valid 8-bit product,
           B[h] = M(h) for h < 256.
  hi == 0: out = I @ B[pos].

Device program (ONE SPMD program on 8 cores, specialized per input):
  - fp32 Taylor scaling-squaring expm -> G, G^T (both chains interleaved,
    elementwise work split DVE / GpSimd)
  - fp32r (full-rate, ~1.6e-4) products build A2/A4/B4 tables, then the
    256-entry A8^T stationary table and the 256-entry B table, rounded
    once to fp16 and kept entirely in SBUF; PSUM evacuations cycle
    DVE/ACT/GpSimd.
  - core id (int32 input) selects one of 8 statically-generated branches
    (nested tc.If/Else); each branch is that core's position loop with
    static table slices: per slot one N=128 fp16 matmul
    psum_quadrant = stat[lo]^T.T @ btab[hi], three-engine-cycled PSUM
    evacuation to fp16, and 2MB output DMAs. No dynamic addressing at all.
Host: dedups positions, sorts by stationary key, splits contiguously
across cores, scatters results back, converts fp16 -> fp32.
"""

import hashlib

import numpy as np
import ml_dtypes

import concourse.bass as bass
import concourse.bacc as bacc
import concourse.mybir as mybir
import concourse.tile as tile
import concourse.tile_utils as tile_utils
tile_utils.max_sbuf_usage = 206 * 1024
from concourse.bass_utils import run_bass_kernel_spmd
from concourse.masks import make_identity

FP = mybir.dt.float32
FR = mybir.dt.float32r
F16 = mybir.dt.float16
I32 = mybir.dt.int32
P = 128
NCORES = 8
S_EXP = 6          # scaling-squaring: X = -H / 2^S_EXP
ORDER = 8          # Taylor order (||H||~37: trunc ~8e-6, fp16 tables dominate)
IDENT_KEY = 256    # stationary key for hi==0 positions
WS = 32            # output wave size in slots

_prog_cache = {}
_last_ctx = None


def _mm(nc, out, lhsT, rhs):
    nc.tensor.matmul(out, lhsT=lhsT, rhs=rhs, start=True, stop=True)


def _build_expm(nc, psump, scratch, praw, ident, g, gt):
    """g[:, b, :] = expm(-H_b) (fp32r), gt[:, b, :] = its transpose.

    The two chains (b=0,1) are interleaved. PSUM-reading elementwise work
    splits DVE (b=0) / ACT (b=1); GpSimd (no PSUM access) takes chain 1's
    SBUF-side adds so no single engine serializes the build.
    """
    copy_f = mybir.ActivationFunctionType.Copy

    def ps_scale(b, dst, src, sc):      # dst = src * sc (src in PSUM)
        if b == 0:
            nc.vector.tensor_scalar_mul(dst, src, sc)
        else:
            nc.scalar.activation(dst, src, copy_f, scale=sc)

    def sb_add(b, dst, a, c):           # dst = a + c (all SBUF)
        (nc.vector if b == 0 else nc.gpsimd).tensor_add(dst, a, c)

    def ps_copy(b, dst, src):           # dst = src (src in PSUM)
        if b == 0:
            nc.vector.tensor_copy(dst, src)
        else:
            nc.scalar.activation(dst, src, copy_f)

    def sb_copy(b, dst, src):           # dst = src (all SBUF)
        (nc.vector if b == 0 else nc.gpsimd).tensor_copy(dst, src)

    xt, t_cur, u_cur = [None, None], [None, None], [None, None]
    for b in range(2):
        pb = praw[:, b, :]
        ps_t = psump.tile([P, 512], FP, tag="pos")
        nc.tensor.transpose(out=ps_t[:, :P], in_=pb, identity=ident[:])
        x = scratch.tile([P, P], FP, tag=f"xt{b}", bufs=1)
        # XT = (P - P^T)/2^s; lhsT=XT gives out = (-H/2^s) @ rhs since H^T=-H
        nc.vector.tensor_tensor(
            out=x[:], in0=pb, in1=ps_t[:, :P], op=mybir.AluOpType.subtract
        )
        nc.vector.tensor_scalar_mul(x[:], x[:], 1.0 / (1 << S_EXP))
        xt[b] = x
        t = scratch.tile([P, P], FP, tag=f"tay{b}")
        sb_copy(b, t[:], ident[:])
        t_cur[b] = t
    for k in range(ORDER, 0, -1):
        for b in range(2):
            ps = psump.tile([P, 512], FP, tag="pos")
            _mm(nc, ps[:, :P], xt[b][:], t_cur[b][:])
            t_nxt = scratch.tile([P, P], FP, tag=f"tay{b}")
            ps_scale(b, t_nxt[:], ps[:, :P], 1.0 / k)
            sb_add(b, t_nxt[:], t_nxt[:], ident[:])
            t_cur[b] = t_nxt
    for b in range(2):
        ps_u = psump.tile([P, 512], FP, tag="pos")
        nc.tensor.transpose(out=ps_u[:, :P], in_=t_cur[b][:], identity=ident[:])
        u = scratch.tile([P, P], FP, tag=f"tayu{b}")
        ps_copy(b, u[:], ps_u[:, :P])
        u_cur[b] = u
    for _ in range(S_EXP):
        for b in range(2):
            ps1 = psump.tile([P, 512], FP, tag="pos")
            ps2 = psump.tile([P, 512], FP, tag="pos")
            _mm(nc, ps1[:, :P], u_cur[b][:], t_cur[b][:])   # T' = T @ T
            _mm(nc, ps2[:, :P], t_cur[b][:], u_cur[b][:])   # U' = (T@T)^T
            t_cur[b] = scratch.tile([P, P], FP, tag=f"tay{b}", name=f"tay{b}")
            u_cur[b] = scratch.tile([P, P], FP, tag=f"tayu{b}", name=f"tayu{b}")
            ps_copy(b, t_cur[b][:], ps1[:, :P])
            ps_copy(b, u_cur[b][:], ps2[:, :P])
    for b in range(2):
        ps_copy(b, g[:, b, :], t_cur[b][:])
        ps_copy(b, gt[:, b, :], u_cur[b][:])


def build_program(core_slots, nslots):
    nc = bacc.Bacc("TRN2", target_bir_lowering=False, debug=False,
                   num_devices=NCORES)
    praw_d = nc.dram_tensor("praw", [2, P, P], FP, kind="ExternalInput")
    cid_d = nc.dram_tensor("cid", [1, 1], I32, kind="ExternalInput")
    out_d = nc.dram_tensor("out", [P, nslots * P], F16, kind="ExternalOutput")

    copy_f = mybir.ActivationFunctionType.Copy
    with tile.TileContext(nc) as tc:
        with (
            tc.tile_pool(name="consts", bufs=1) as consts,
            tc.tile_pool(name="scratch", bufs=2) as scratch,
            tc.tile_pool(name="obuf", bufs=3) as obufp,
            tc.tile_pool(name="psum", bufs=8, space="PSUM") as psump,
        ):
            ident = consts.tile([P, P], FP, tag="ident")
            make_identity(nc, ident[:])
            identh = consts.tile([P, P], F16, tag="identh")
            nc.vector.tensor_copy(identh[:], ident[:])
            identr = consts.tile([P, P], FR, tag="identr")
            nc.vector.tensor_copy(identr[:], ident[:])
            praw = consts.tile([P, 2, P], FP, tag="praw")
            nc.sync.dma_start(praw[:], praw_d[:].rearrange("p r c -> r p c"))
            cidt = consts.tile([1, 1], I32, tag="cid")
            nc.sync.dma_start(cidt[:], cid_d[:])

            g = consts.tile([P, 2, P], FR, tag="g")
            gt = consts.tile([P, 2, P], FR, tag="gt")
            a2 = consts.tile([P, 4, P], FR, tag="a2")
            a2t = consts.tile([P, 4, P], FR, tag="a2t")
            a4 = consts.tile([P, 16, P], FR, tag="a4")
            a4t = consts.tile([P, 16, P], FR, tag="a4t")
            b4 = consts.tile([P, 16, P], FR, tag="b4")
            btab = consts.tile([P, 256, P], F16, tag="btab")
            stab = consts.tile([P, 256, P], F16, tag="stab")

            # fp16 PSUM evacuations alternate DVE / ACT (GpSimd cannot
            # access PSUM); fp32r ones stay on DVE
            def evac16(i, dst, src):
                if i % 2 == 0:
                    nc.vector.tensor_copy(dst, src)
                else:
                    nc.scalar.activation(dst, src, copy_f)

            # ---- phase A: primitives (fp32 Taylor) ----
            _build_expm(nc, psump, scratch, praw, ident, g, gt)

            gf = g[:].rearrange("r m c -> r (m c)")
            gtf = gt[:].rearrange("r m c -> r (m c)")

            # ---- phase B: A2/A2T/A4/A4T (fp32r, N>=256 full rate) ----
            for a in range(2):
                ps = psump.tile([P, 512], FP, tag="pos")
                _mm(nc, ps[:, :256], gt[:, a, :], gf)      # [A2[a], A2[a+2]]
                nc.vector.tensor_copy(
                    a2[:, a : a + 3 : 2, :],
                    ps[:, :256].rearrange("r (m c) -> r m c", c=P),
                )
            for b in range(2):
                ps = psump.tile([P, 512], FP, tag="pos")
                _mm(nc, ps[:, :256], g[:, b, :], gtf)      # [A2T[2b], A2T[2b+1]]
                nc.vector.tensor_copy(
                    a2t[:, 2 * b : 2 * b + 2, :],
                    ps[:, :256].rearrange("r (m c) -> r m c", c=P),
                )
            a2f = a2[:].rearrange("r m c -> r (m c)")
            a2tf = a2t[:].rearrange("r m c -> r (m c)")
            for a in range(4):
                ps = psump.tile([P, 512], FP, tag="pos")
                _mm(nc, ps[:], a2t[:, a, :], a2f)          # A4[a+4g] over g
                nc.vector.tensor_copy(
                    a4[:, a : a + 13 : 4, :],
                    ps[:].rearrange("r (m c) -> r m c", c=P),
                )
            for gg in range(4):
                ps = psump.tile([P, 512], FP, tag="pos")
                _mm(nc, ps[:], a2[:, gg, :], a2tf)         # A4T[4g+j] over j
                nc.vector.tensor_copy(
                    a4t[:, 4 * gg : 4 * gg + 4, :],
                    ps[:].rearrange("r (m c) -> r m c", c=P),
                )

            # ---- phase C: B4 table (M(h), h<16) ----
            nc.vector.tensor_copy(b4[:, 0, :], identr[:])
            nc.vector.tensor_copy(b4[:, 1, :], identr[:])
            nc.vector.tensor_copy(b4[:, 2, :], g[:, 0, :])
            nc.vector.tensor_copy(b4[:, 3, :], g[:, 1, :])
            for b in range(2):
                ps = psump.tile([P, 512], FP, tag="pos")
                _mm(nc, ps[:, :256], gt[:, b, :],
                    b4[:, 2:4, :].rearrange("r m c -> r (m c)"))
                # [B4[4+b], B4[6+b]]
                nc.vector.tensor_copy(
                    b4[:, 4 + b : 8 : 2, :],
                    ps[:, :256].rearrange("r (m c) -> r m c", c=P),
                )
            for b in range(2):
                ps = psump.tile([P, 512], FP, tag="pos")
                _mm(nc, ps[:], gt[:, b, :],
                    b4[:, 4:8, :].rearrange("r m c -> r (m c)"))
                # [B4[8+b], B4[10+b], B4[12+b], B4[14+b]]
                nc.vector.tensor_copy(
                    b4[:, 8 + b : 16 : 2, :],
                    ps[:].rearrange("r (m c) -> r m c", c=P),
                )

            # ---- phase D: stationary table A8T fp16 (256 entries, SBUF) ----
            # (before the B table: the position loops wait on btab's last
            #  write, so finish stab first)
            a4tf = a4t[:].rearrange("r m c -> r (m c)")
            ei = 0
            for gg in range(16):
                for q in range(4):
                    ps = psump.tile([P, 512], FP, tag="pos")
                    _mm(nc, ps[:], a4[:, gg, :], a4tf[:, q * 512 : (q + 1) * 512])
                    evac16(
                        ei,
                        stab[:, 16 * gg + 4 * q : 16 * gg + 4 * q + 4, :],
                        ps[:].rearrange("r (m c) -> r m c", c=P),
                    )
                    ei += 1

            # ---- phase E: B table fp16 (256 entries, SBUF) ----
            b4f = b4[:].rearrange("r m c -> r (m c)")
            nc.scalar.activation(
                btab[:, 0:16, :].rearrange("r m c -> r (m c)"), b4f, copy_f
            )
            for m in range(16):
                for q in range(4):
                    ps = psump.tile([P, 512], FP, tag="pos")
                    _mm(nc, ps[:], a4t[:, m, :], b4f[:, q * 512 : (q + 1) * 512])
                    # entries m+16c, c=4q..4q+3 (skip c==0: h<16 handled above)
                    if q == 0:
                        evac16(
                            ei,
                            btab[:, m + 16 : m + 49 : 16, :],
                            ps[:, 128:512].rearrange("r (m c) -> r m c", c=P),
                        )
                    else:
                        c0 = m + 64 * q
                        evac16(
                            ei,
                            btab[:, c0 : c0 + 49 : 16, :],
                            ps[:].rearrange("r (m c) -> r m c", c=P),
                        )
                    ei += 1

            # ---- phase F: per-core position loops (static, branch on cid) ----
            cid = nc.values_load(cidt[0:1, 0:1], min_val=0, max_val=NCORES - 1,
                                 skip_runtime_bounds_check=True)

            def body(c):
                slots = core_slots[c]
                nq = nslots // 4
                for w0 in range(0, nq, WS // 4):
                    wq = min(WS // 4, nq - w0)
                    ob = obufp.tile([P, WS * P], F16, tag="ob")
                    for qi in range(wq):
                        q = w0 + qi
                        ps = psump.tile([P, 512], FP, tag="pos")
                        for j in range(4):
                            key, bent = slots[4 * q + j]
                            lhsT = identh[:] if key == IDENT_KEY else stab[:, key, :]
                            nc.tensor.matmul(
                                ps[:, j * P : (j + 1) * P],
                                lhsT=lhsT,
                                rhs=btab[:, bent, :],
                                start=True,
                                stop=True,
                            )
                        evac16(qi, ob[:, qi * 512 : (qi + 1) * 512], ps[:])
                    nc.sync.dma_start(
                        out_d[:, w0 * 4 * P : (w0 * 4 + wq * 4) * P],
                        ob[:, : wq * 4 * P],
                    )

            def emit(c):
                if c == NCORES - 1:
                    body(c)
                    return
                with tc.If(cid == c) as cmp:
                    body(c)
                with cmp.Else():
                    emit(c + 1)

            emit(0)
    nc.compile()
    return nc


def _plan(unique):
    """Dedup + per-core static slot lists sorted by stationary key."""
    uq = np.asarray(unique).astype(np.int64)
    dvals, dinv = np.unique(uq, return_inverse=True)
    lo = dvals & 255
    hi = dvals >> 8
    key = np.where(hi > 0, lo, IDENT_KEY)
    bent = np.where(hi > 0, hi, dvals)
    order = np.argsort(key, kind="stable")
    n = len(dvals)
    bounds = [(c * n) // NCORES for c in range(NCORES + 1)]
    nslots = -(-max(bounds[c + 1] - bounds[c] for c in range(NCORES)) // 4) * 4
    core_slots = []
    gslot = np.zeros(n, np.int64)
    for c in range(NCORES):
        idxs = order[bounds[c] : bounds[c + 1]]
        sl = [(int(key[i]), int(bent[i])) for i in idxs]
        gslot[idxs] = c * nslots + np.arange(len(sl))
        sl += [(IDENT_KEY, 0)] * (nslots - len(sl))
        core_slots.append(sl)
    return core_slots, nslots, gslot, dinv


def kernel(unique, primitives_raw, identity=None, **_):
    unique = np.asarray(unique)
    praw = np.ascontiguousarray(np.asarray(primitives_raw, np.float32))

    ckey = hashlib.sha256(unique.tobytes()).hexdigest()
    if ckey not in _prog_cache:
        core_slots, nslots, gslot, dinv = _plan(unique)
        nc = build_program(core_slots, nslots)
        _prog_cache.clear()
        _prog_cache[ckey] = (nc, nslots, gslot, dinv)
    nc, nslots, gslot, dinv = _prog_cache[ckey]

    in_maps = [
        {"praw": praw, "cid": np.array([[c]], np.int32)}
        for c in range(NCORES)
    ]
    global _last_ctx
    _last_ctx = (nc, in_maps)
    res = run_bass_kernel_spmd(nc, in_maps, list(range(NCORES)))
    outs = np.concatenate(
        [
            np.asarray(res.results[c]["out"])
            .reshape(P, nslots, P)
            .transpose(1, 0, 2)
            for c in range(NCORES)
        ],
        axis=0,
    )
    return outs[gslot[dinv]].astype(np.float32)


if __name__ == "__main__":
    rng = np.random.default_rng(0)
    u = rng.integers(1, 65536, 256).astype(np.int32)
    pr = rng.random((2, P, P), np.float32)
    o = kernel(u, pr)
    print(o.shape, o.dtype)


# revision 9
# speedup vs baseline: 6.2400x; 1.0006x over previous
"""Trainium2 Bass kernel for nn_BinaryPathEncoder.

Math: out[n] = prod_k G_{b_k(pos_n)} over the binary digits of pos_n below
its leading 1 (LSB first, leftmost), where G_b = expm(-(P_b - P_b^T)).

Decomposition (pos = hi*256 + lo):
  hi >= 1: out = A8(lo) @ B[hi], A8(lo) = all-valid 8-bit product,
           B[h] = M(h) for h < 256.
  hi == 0: out = I @ B[pos].

Device program (ONE SPMD program on 8 cores, specialized per input):
  - fp32 Taylor scaling-squaring expm -> G, G^T (both chains interleaved,
    elementwise work split DVE / GpSimd)
  - fp32r (full-rate, ~1.6e-4) products build A2/A4/B4 tables, then the
    256-entry A8^T stationary table and the 256-entry B table, rounded
    once to fp16 and kept entirely in SBUF; PSUM evacuations cycle
    DVE/ACT/GpSimd.
  - core id (int32 input) selects one of 8 statically-generated branches
    (nested tc.If/Else); each branch is that core's position loop with
    static table slices: per slot one N=128 fp16 matmul
    psum_quadrant = stat[lo]^T.T @ btab[hi], three-engine-cycled PSUM
    evacuation to fp16, and 2MB output DMAs. No dynamic addressing at all.
Host: dedups positions, sorts by stationary key, splits contiguously
across cores, scatters results back, converts fp16 -> fp32.
"""

import hashlib

import numpy as np
import ml_dtypes

import concourse.bass as bass
import concourse.bacc as bacc
import concourse.mybir as mybir
import concourse.tile as tile
import concourse.tile_utils as tile_utils
tile_utils.max_sbuf_usage = 206 * 1024
from concourse.bass_utils import run_bass_kernel_spmd
from concourse.masks import make_identity

FP = mybir.dt.float32
FR = mybir.dt.float32r
F16 = mybir.dt.float16
I32 = mybir.dt.int32
P = 128
NCORES = 8
S_EXP = 6          # scaling-squaring: X = -H / 2^S_EXP
ORDER = 8          # Taylor order (||H||~37: trunc ~8e-6, fp16 tables dominate)
IDENT_KEY = 256    # stationary key for hi==0 positions
WS = 32            # output wave size in slots

_prog_cache = {}
_last_ctx = None


def _mm(nc, out, lhsT, rhs):
    nc.tensor.matmul(out, lhsT=lhsT, rhs=rhs, start=True, stop=True)


def _build_expm(nc, psump, scratch, praw, ident, g, gt):
    """g[:, b, :] = expm(-H_b) (fp32r), gt[:, b, :] = its transpose.

    The two chains (b=0,1) are interleaved. PSUM-reading elementwise work
    splits DVE (b=0) / ACT (b=1); GpSimd (no PSUM access) takes chain 1's
    SBUF-side adds so no single engine serializes the build.
    """
    copy_f = mybir.ActivationFunctionType.Copy

    def ps_copy(b, dst, src):           # dst = src (src in PSUM)
        if b == 0:
            nc.vector.tensor_copy(dst, src)
        else:
            nc.scalar.activation(dst, src, copy_f)

    def sb_copy(b, dst, src):           # dst = src (all SBUF)
        (nc.vector if b == 0 else nc.gpsimd).tensor_copy(dst, src)

    xt, t_cur, u_cur = [None, None], [None, None], [None, None]
    for b in range(2):
        pb = praw[:, b, :]
        ps_t = psump.tile([P, 512], FP, tag="pos")
        nc.tensor.transpose(out=ps_t[:, :P], in_=pb, identity=ident[:])
        x = scratch.tile([P, P], FP, tag=f"xt{b}", bufs=1)
        # XT = (P - P^T)/2^s; lhsT=XT gives out = (-H/2^s) @ rhs since H^T=-H
        nc.vector.tensor_tensor(
            out=x[:], in0=pb, in1=ps_t[:, :P], op=mybir.AluOpType.subtract
        )
        nc.vector.tensor_scalar_mul(x[:], x[:], 1.0 / (1 << S_EXP))
        xt[b] = x
        t = scratch.tile([P, P], FP, tag=f"tay{b}")
        sb_copy(b, t[:], ident[:])
        t_cur[b] = t
    for k in range(ORDER, 0, -1):
        for b in range(2):
            ps = psump.tile([P, 512], FP, tag="pos")
            _mm(nc, ps[:, :P], xt[b][:], t_cur[b][:])
            t_nxt = scratch.tile([P, P], FP, tag=f"tay{b}")
            # t_nxt = ps/k + I in one DVE op
            nc.vector.scalar_tensor_tensor(
                t_nxt[:], ps[:, :P], 1.0 / k, ident[:],
                op0=mybir.AluOpType.mult, op1=mybir.AluOpType.add,
            )
            t_cur[b] = t_nxt
    for b in range(2):
        ps_u = psump.tile([P, 512], FP, tag="pos")
        nc.tensor.transpose(out=ps_u[:, :P], in_=t_cur[b][:], identity=ident[:])
        u = scratch.tile([P, P], FP, tag=f"tayu{b}")
        ps_copy(b, u[:], ps_u[:, :P])
        u_cur[b] = u
    for _ in range(S_EXP):
        for b in range(2):
            ps1 = psump.tile([P, 512], FP, tag="pos")
            ps2 = psump.tile([P, 512], FP, tag="pos")
            _mm(nc, ps1[:, :P], u_cur[b][:], t_cur[b][:])   # T' = T @ T
            _mm(nc, ps2[:, :P], t_cur[b][:], u_cur[b][:])   # U' = (T@T)^T
            t_cur[b] = scratch.tile([P, P], FP, tag=f"tay{b}", name=f"tay{b}")
            u_cur[b] = scratch.tile([P, P], FP, tag=f"tayu{b}", name=f"tayu{b}")
            ps_copy(b, t_cur[b][:], ps1[:, :P])
            ps_copy(b, u_cur[b][:], ps2[:, :P])
    for b in range(2):
        ps_copy(b, g[:, b, :], t_cur[b][:])
        ps_copy(b, gt[:, b, :], u_cur[b][:])


def build_program(core_slots, nslots):
    nc = bacc.Bacc("TRN2", target_bir_lowering=False, debug=False,
                   num_devices=NCORES)
    praw_d = nc.dram_tensor("praw", [2, P, P], FP, kind="ExternalInput")
    cid_d = nc.dram_tensor("cid", [1, 1], I32, kind="ExternalInput")
    out_d = nc.dram_tensor("out", [P, nslots * P], F16, kind="ExternalOutput")

    copy_f = mybir.ActivationFunctionType.Copy
    with tile.TileContext(nc) as tc:
        with (
            tc.tile_pool(name="consts", bufs=1) as consts,
            tc.tile_pool(name="scratch", bufs=2) as scratch,
            tc.tile_pool(name="obuf", bufs=3) as obufp,
            tc.tile_pool(name="psum", bufs=8, space="PSUM") as psump,
        ):
            ident = consts.tile([P, P], FP, tag="ident")
            make_identity(nc, ident[:])
            identh = consts.tile([P, P], F16, tag="identh")
            nc.vector.tensor_copy(identh[:], ident[:])
            identr = consts.tile([P, P], FR, tag="identr")
            nc.vector.tensor_copy(identr[:], ident[:])
            praw = consts.tile([P, 2, P], FP, tag="praw")
            nc.sync.dma_start(praw[:], praw_d[:].rearrange("p r c -> r p c"))
            cidt = consts.tile([1, 1], I32, tag="cid")
            nc.sync.dma_start(cidt[:], cid_d[:])

            g = consts.tile([P, 2, P], FR, tag="g")
            gt = consts.tile([P, 2, P], FR, tag="gt")
            a2 = consts.tile([P, 4, P], FR, tag="a2")
            a2t = consts.tile([P, 4, P], FR, tag="a2t")
            a4 = consts.tile([P, 16, P], FR, tag="a4")
            a4t = consts.tile([P, 16, P], FR, tag="a4t")
            b4 = consts.tile([P, 16, P], FR, tag="b4")
            btab = consts.tile([P, 256, P], F16, tag="btab")
            stab = consts.tile([P, 256, P], F16, tag="stab")

            # fp16 PSUM evacuations alternate DVE / ACT (GpSimd cannot
            # access PSUM); fp32r ones stay on DVE
            def evac16(i, dst, src):
                if i % 2 == 0:
                    nc.vector.tensor_copy(dst, src)
                else:
                    nc.scalar.activation(dst, src, copy_f)

            # ---- phase A: primitives (fp32 Taylor) ----
            _build_expm(nc, psump, scratch, praw, ident, g, gt)

            gf = g[:].rearrange("r m c -> r (m c)")
            gtf = gt[:].rearrange("r m c -> r (m c)")

            # ---- phase B: A2/A2T/A4/A4T (fp32r, N>=256 full rate) ----
            for a in range(2):
                ps = psump.tile([P, 512], FP, tag="pos")
                _mm(nc, ps[:, :256], gt[:, a, :], gf)      # [A2[a], A2[a+2]]
                nc.vector.tensor_copy(
                    a2[:, a : a + 3 : 2, :],
                    ps[:, :256].rearrange("r (m c) -> r m c", c=P),
                )
            for b in range(2):
                ps = psump.tile([P, 512], FP, tag="pos")
                _mm(nc, ps[:, :256], g[:, b, :], gtf)      # [A2T[2b], A2T[2b+1]]
                nc.vector.tensor_copy(
                    a2t[:, 2 * b : 2 * b + 2, :],
                    ps[:, :256].rearrange("r (m c) -> r m c", c=P),
                )
            a2f = a2[:].rearrange("r m c -> r (m c)")
            a2tf = a2t[:].rearrange("r m c -> r (m c)")
            for a in range(4):
                ps = psump.tile([P, 512], FP, tag="pos")
                _mm(nc, ps[:], a2t[:, a, :], a2f)          # A4[a+4g] over g
                nc.vector.tensor_copy(
                    a4[:, a : a + 13 : 4, :],
                    ps[:].rearrange("r (m c) -> r m c", c=P),
                )
            for gg in range(4):
                ps = psump.tile([P, 512], FP, tag="pos")
                _mm(nc, ps[:], a2[:, gg, :], a2tf)         # A4T[4g+j] over j
                nc.vector.tensor_copy(
                    a4t[:, 4 * gg : 4 * gg + 4, :],
                    ps[:].rearrange("r (m c) -> r m c", c=P),
                )

            # ---- phase C: B4 table (M(h), h<16) ----
            nc.vector.tensor_copy(b4[:, 0, :], identr[:])
            nc.vector.tensor_copy(b4[:, 1, :], identr[:])
            nc.vector.tensor_copy(b4[:, 2, :], g[:, 0, :])
            nc.vector.tensor_copy(b4[:, 3, :], g[:, 1, :])
            for b in range(2):
                ps = psump.tile([P, 512], FP, tag="pos")
                _mm(nc, ps[:, :256], gt[:, b, :],
                    b4[:, 2:4, :].rearrange("r m c -> r (m c)"))
                # [B4[4+b], B4[6+b]]
                nc.vector.tensor_copy(
                    b4[:, 4 + b : 8 : 2, :],
                    ps[:, :256].rearrange("r (m c) -> r m c", c=P),
                )
            for b in range(2):
                ps = psump.tile([P, 512], FP, tag="pos")
                _mm(nc, ps[:], gt[:, b, :],
                    b4[:, 4:8, :].rearrange("r m c -> r (m c)"))
                # [B4[8+b], B4[10+b], B4[12+b], B4[14+b]]
                nc.vector.tensor_copy(
                    b4[:, 8 + b : 16 : 2, :],
                    ps[:].rearrange("r (m c) -> r m c", c=P),
                )

            # ---- phase D: B table fp16 (256 entries, SBUF) ----
            # (built before the stationary table: every position-loop quad
            #  waits on btab, but only on its own key's stab entry, so the
            #  loop overlaps the stab build)
            ei = 0
            b4f = b4[:].rearrange("r m c -> r (m c)")
            nc.scalar.activation(
                btab[:, 0:16, :].rearrange("r m c -> r (m c)"), b4f, copy_f
            )
            for m in range(16):
                for q in range(4):
                    ps = psump.tile([P, 512], FP, tag="pos")
                    _mm(nc, ps[:], a4t[:, m, :], b4f[:, q * 512 : (q + 1) * 512])
                    # entries m+16c, c=4q..4q+3 (skip c==0: h<16 handled above)
                    if q == 0:
                        evac16(
                            ei,
                            btab[:, m + 16 : m + 49 : 16, :],
                            ps[:, 128:512].rearrange("r (m c) -> r m c", c=P),
                        )
                    else:
                        c0 = m + 64 * q
                        evac16(
                            ei,
                            btab[:, c0 : c0 + 49 : 16, :],
                            ps[:].rearrange("r (m c) -> r m c", c=P),
                        )
                    ei += 1

            # ---- phase E: stationary table A8T fp16 (256 entries, SBUF),
            #      key-ascending so loop quads unblock progressively ----
            a4tf = a4t[:].rearrange("r m c -> r (m c)")
            for gg in range(16):
                for q in range(4):
                    ps = psump.tile([P, 512], FP, tag="pos")
                    _mm(nc, ps[:], a4[:, gg, :], a4tf[:, q * 512 : (q + 1) * 512])
                    evac16(
                        ei,
                        stab[:, 16 * gg + 4 * q : 16 * gg + 4 * q + 4, :],
                        ps[:].rearrange("r (m c) -> r m c", c=P),
                    )
                    ei += 1

            # ---- phase F: per-core position loops (static, branch on cid) ----
            cid = nc.values_load(cidt[0:1, 0:1], min_val=0, max_val=NCORES - 1,
                                 skip_runtime_bounds_check=True)

            def body(c):
                slots = core_slots[c]
                nq = nslots // 4
                for w0 in range(0, nq, WS // 4):
                    wq = min(WS // 4, nq - w0)
                    ob = obufp.tile([P, WS * P], F16, tag="ob")
                    for qi in range(wq):
                        q = w0 + qi
                        ps = psump.tile([P, 512], FP, tag="pos")
                        for j in range(4):
                            key, bent = slots[4 * q + j]
                            lhsT = identh[:] if key == IDENT_KEY else stab[:, key, :]
                            nc.tensor.matmul(
                                ps[:, j * P : (j + 1) * P],
                                lhsT=lhsT,
                                rhs=btab[:, bent, :],
                                start=True,
                                stop=True,
                            )
                        evac16(qi, ob[:, qi * 512 : (qi + 1) * 512], ps[:])
                    nc.sync.dma_start(
                        out_d[:, w0 * 4 * P : (w0 * 4 + wq * 4) * P],
                        ob[:, : wq * 4 * P],
                    )

            def emit(c):
                if c == NCORES - 1:
                    body(c)
                    return
                with tc.If(cid == c) as cmp:
                    body(c)
                with cmp.Else():
                    emit(c + 1)

            emit(0)
    nc.compile()
    return nc


def _plan(unique):
    """Dedup + per-core static slot lists sorted by stationary key."""
    uq = np.asarray(unique).astype(np.int64)
    dvals, dinv = np.unique(uq, return_inverse=True)
    lo = dvals & 255
    hi = dvals >> 8
    key = np.where(hi > 0, lo, IDENT_KEY)
    bent = np.where(hi > 0, hi, dvals)
    order = np.argsort(key, kind="stable")
    n = len(dvals)
    bounds = [(c * n) // NCORES for c in range(NCORES + 1)]
    nslots = -(-max(bounds[c + 1] - bounds[c] for c in range(NCORES)) // 8) * 8
    core_slots = []
    gslot = np.zeros(n, np.int64)
    for c in range(NCORES):
        idxs = order[bounds[c] : bounds[c + 1]]
        sl = [(int(key[i]), int(bent[i])) for i in idxs]
        gslot[idxs] = c * nslots + np.arange(len(sl))
        sl += [(IDENT_KEY, 0)] * (nslots - len(sl))
        core_slots.append(sl)
    return core_slots, nslots, gslot, dinv


def kernel(unique, primitives_raw, identity=None, **_):
    unique = np.asarray(unique)
    praw = np.ascontiguousarray(np.asarray(primitives_raw, np.float32))

    ckey = hashlib.sha256(unique.tobytes()).hexdigest()
    if ckey not in _prog_cache:
        core_slots, nslots, gslot, dinv = _plan(unique)
        nc = build_program(core_slots, nslots)
        _prog_cache.clear()
        _prog_cache[ckey] = (nc, nslots, gslot, dinv)
    nc, nslots, gslot, dinv = _prog_cache[ckey]

    in_maps = [
        {"praw": praw, "cid": np.array([[c]], np.int32)}
        for c in range(NCORES)
    ]
    global _last_ctx
    _last_ctx = (nc, in_maps)
    res = run_bass_kernel_spmd(nc, in_maps, list(range(NCORES)))
    outs = np.concatenate(
        [
            np.asarray(res.results[c]["out"])
            .reshape(P, nslots, P)
            .transpose(1, 0, 2)
            for c in range(NCORES)
        ],
        axis=0,
    )
    return outs[gslot[dinv]].astype(np.float32)


if __name__ == "__main__":
    rng = np.random.default_rng(0)
    u = rng.integers(1, 65536, 256).astype(np.int32)
    pr = rng.random((2, P, P), np.float32)
    o = kernel(u, pr)
    print(o.shape, o.dtype)
